# revision 19
# baseline (speedup 1.0000x reference)
"""Locally-connected layer (3x3, stride 1) on 8 TRN2 NeuronCores — v2.

Math: out[b,o,i,j] = sum_{c,kh,kw} x[b,c,i+kh,j+kw] * W[c,o,i,j,kh,kw] + bias[o,i,j]
  x: [128, 64, 32, 32] f32, W: [64, 64, 30, 30, 3, 3] f32, bias: [64, 30, 30] f32
  out: [128, 64, 30, 30] f32

Sharding: each core owns 4 output rows (cores 6,7 overlap rows so all cores run
an identical program; host keeps rows 28-29 from core 7).

Per-core schedule (data bf16, PSUM f32):
- Contract (c, kh-pair): SBUF x tile packs channels of pixel rows (h, h+1) on
  partitions (0-63, 64-127).  A "pair" matmul contracts kh=(0,1) for output row
  i=h in one 128-deep pass; kh=2 is a 64-deep "single" matmul against one half
  of the same loaded stationary (row-group select via base partition).
- Diagonal weight layout: for stationary x-column w, one matmul covers output
  columns j=w-2..w (kw=2-t), N up to 192.  Matmuls split at PSUM bank edges.
- PSUM: one bank per (row, j-band of 8).  start=True on the first matmul into a
  bank clears its has_written bits; per-element first touch then overwrites.
- Eviction: DVE (psum + bias) -> bf16 out staging; 2 contiguous out DMAs.
  Host upcasts to f32.

Sweep order (h = x slab, pairs (h,h+1)): h=0 pair(i=0); h=1 pair(i=1) +
single(i=0, bottom); h=2 pair(i=2); h=3 singles(i=1 top, i=2 bottom);
h=3 pair(i=3); h=5 single(i=3, top).  Keeps <= 8 PSUM banks live.
"""

import sys

for _p in ("/opt/trn_rl_repo",):
    if _p not in sys.path:
        sys.path.insert(0, _p)

import numpy as np
import ml_dtypes

import concourse.bass as bass
import concourse.tile as tile
from concourse import bacc, mybir
from concourse.bass_utils import run_bass_kernel_spmd

N_CORES = 8
B = 128
C = 64
O = 64
H = 32
W = 32
K = 3
OH = 30
OW = 30
R = 4
ROW0 = [0, 4, 8, 12, 16, 20, 24, 26]

NSLAB = 5                      # x slabs at h = 0,1,2,3,5 (pair = rows (h, h+1))
SLABS = [0, 1, 2, 3, 5]
XT_FREE = NSLAB * W * B        # 20480 elems / partition
OUT_FREE = R * OW * O          # 7680

# tight-packed (w, t) weight layout: only slots with a valid output column
# j = w-2+t are stored; per-w start offsets and t-ranges:
_T_LO = [max(0, 2 - w) for w in range(W)]
_T_HI = [min(K - 1, (OH - 1) - (w - 2)) for w in range(W)]
_NT = [hi - lo + 1 for lo, hi in zip(_T_LO, _T_HI)]
_OFFW = np.concatenate([[0], np.cumsum(_NT)]).astype(int)
WCH = int(_OFFW[-1]) * O       # 5760: one weight chunk (packed (w,t), o)

_BF16 = ml_dtypes.bfloat16
STAGGERED = True   # staggered semaphore reset on the timing loop back-edge


def _bank_split(j_lo, j_hi):
    """Split [j_lo, j_hi] into (a, b, bank) within-bank runs."""
    segs = []
    a = j_lo
    while a <= j_hi:
        bk = a // 8
        b = min(j_hi, bk * 8 + 7)
        segs.append((a, b, bk))
        a = b + 1
    return segs


def _segments(w, kind):
    """(a, b, bank, fresh) matmul output runs for stationary column w.

    A pair sweep writes column j=w for the first time (fresh: overwrite path,
    start=True at bank starts) while j=w-2..w-1 accumulate; each matmul's
    region must be homogeneous, so fresh and old are separate matmuls.
    Single sweeps always accumulate (the row's pair sweep ran first)."""
    j_lo, j_hi = max(0, w - 2), min(OH - 1, w)
    if j_lo > j_hi:
        return []
    if kind == "pair":
        segs = []
        if w >= 1:
            segs += [(a, b, bk, False) for (a, b, bk)
                     in _bank_split(j_lo, min(OH - 1, w - 1))]
        if w <= OH - 1:
            segs.append((w, w, w // 8, True))
        return segs
    return [(a, b, bk, False) for (a, b, bk) in _bank_split(j_lo, j_hi)]


def build_nc(repeat: int = 1, mode: str = "full"):
    nc = bacc.Bacc("TRN2", target_bir_lowering=False, debug=False,
                   num_devices=N_CORES)
    xt_ap = nc.dram_tensor("xt", [B, XT_FREE], mybir.dt.bfloat16,
                           kind="ExternalInput").ap()
    wp_ap = nc.dram_tensor("wp", [B, R * WCH], mybir.dt.bfloat16,
                           kind="ExternalInput").ap()
    ws_ap = nc.dram_tensor("ws", [B, 2 * WCH], mybir.dt.bfloat16,
                           kind="ExternalInput").ap()
    bias_ap = nc.dram_tensor("biasb", [B, OUT_FREE], mybir.dt.float32,
                             kind="ExternalInput").ap()
    out_ap = nc.dram_tensor("outp", [B, OUT_FREE], mybir.dt.bfloat16,
                            kind="ExternalOutput").ap()

    with tile.TileContext(nc) as tc:
        with (
            tc.tile_pool(name="xpool", bufs=1) as xpool,
            tc.tile_pool(name="bpool", bufs=1) as bpool,
            tc.tile_pool(name="wppool", bufs=4) as wppool,
            tc.tile_pool(name="wspool", bufs=2) as wspool,
            tc.tile_pool(name="ppool", bufs=8, space="PSUM") as ppool,
            tc.tile_pool(name="opool", bufs=2) as opool,
        ):
            xt_sb = xpool.tile([B, XT_FREE], mybir.dt.bfloat16)
            nc.scalar.dma_start(xt_sb[:, :XT_FREE // 2], xt_ap[:, :XT_FREE // 2])
            nc.scalar.dma_start(xt_sb[:, XT_FREE // 2:], xt_ap[:, XT_FREE // 2:])
            bias_sb = bpool.tile([B, OUT_FREE], mybir.dt.float32)
            nc.scalar.dma_start(bias_sb, bias_ap)
            x4 = xt_sb[:].rearrange("p (h w b) -> p h w b", w=W, b=B)

            FULL, TOP, BOT = (0, B), (0, C), (C, B)

            def body():
                if mode == "empty":
                    return
                wp, ws = {}, {}

                def load_wp(i):
                    t = wppool.tile([B, WCH], mybir.dt.bfloat16, tag="wp")
                    nc.sync.dma_start(t, wp_ap[:, i * WCH:(i + 1) * WCH])
                    wp[i] = t

                def load_ws(s):
                    t = wspool.tile([B, WCH], mybir.dt.bfloat16, tag="ws")
                    nc.sync.dma_start(t, ws_ap[:, s * WCH:(s + 1) * WCH])
                    ws[s] = t

                load_wp(0)
                load_wp(1)
                load_ws(0)
                load_wp(2)
                load_ws(1)
                load_wp(3)

                ps = {}
                out_sb = opool.tile([B, OUT_FREE], mybir.dt.bfloat16)

                def evict(i, bk):
                    off = i * (OW * O) + bk * 8 * O
                    n = (min(OH, bk * 8 + 8) - bk * 8) * O
                    nc.vector.scalar_tensor_tensor(
                        out_sb[:, off:off + n], ps[(i, bk)][:, :n], 1.0,
                        bias_sb[:, off:off + n],
                        op0=mybir.AluOpType.mult, op1=mybir.AluOpType.add)
                    ps.pop((i, bk))

                def sweep(slab, entries):
                    """entries: list of (kind, i, (lo, hi), w-view); emitted
                    interleaved per w so they share the loaded stationary."""
                    for w in range(W):
                        for (kind, i, (lo, hi), wv) in entries:
                            lhsT = x4[lo:hi, slab, w, :]
                            for (a, b, bk, fresh) in _segments(w, kind):
                                t0 = a - (w - 2)
                                nt = b - a + 1
                                woff = (int(_OFFW[w]) + t0 - _T_LO[w]) * O
                                rhs = wv[lo:hi, woff:woff + nt * O]
                                start = fresh and w == 8 * bk
                                if start:
                                    ps[(i, bk)] = ppool.tile(
                                        [B, 512], mybir.dt.float32, tag="ps",
                                        name=f"ps{i}_{bk}")
                                stop = (kind == "single"
                                        and ((bk < 3 and w == 8 * bk + 9)
                                             or (bk == 3 and w == W - 1)))
                                dst = ps[(i, bk)][:, (a - 8 * bk) * O:
                                                  (b + 1 - 8 * bk) * O]
                                nc.tensor.matmul(dst, lhsT, rhs,
                                                 start=start, stop=stop)
                                if stop:
                                    evict(i, bk)

                if mode != "nomm":
                    sweep(0, [("pair", 0, FULL, wp[0])])
                    sweep(1, [("pair", 1, FULL, wp[1]),
                              ("single", 0, BOT, ws[0])])
                    sweep(2, [("pair", 2, FULL, wp[2])])
                    sweep(3, [("single", 1, TOP, ws[0]),
                              ("single", 2, BOT, ws[1])])
                    sweep(3, [("pair", 3, FULL, wp[3])])
                    sweep(4, [("single", 3, TOP, ws[1])])

                nc.scalar.dma_start(out_ap[:, :OUT_FREE // 2],
                                    out_sb[:, :OUT_FREE // 2])
                nc.scalar.dma_start(out_ap[:, OUT_FREE // 2:],
                                    out_sb[:, OUT_FREE // 2:])

            if repeat == 1:
                body()
            else:
                with tc.For_i(0, repeat, 1,
                              hint_engines=(mybir.EngineType.PE,),
                              staggered_reset=STAGGERED):
                    body()

    nc.compile()
    dedup_ldweights(nc)
    return nc


def _ldw_desc(inst):
    """(memref, free_offset, pitch, p0, np, free_ap, dtype) of a LdW, or None."""
    try:
        ap = inst.ins[0]
        pitch, npart = ap.ap[0]
        p0 = ap.bass_ap.base_partition()
        free = tuple(tuple(d) for d in list(ap.ap)[1:])
        return (str(ap.memref), int(ap.offset) - p0 * int(pitch), int(pitch),
                int(p0), int(npart), free, ap.dtype)
    except Exception:
        return None


def dedup_ldweights(nc):
    """Remove InstLdweights that reload PE rows already holding the same data.

    The previous kept LdW loaded partitions [p0, p0+n0) of (memref, offset,
    free pattern); a following LdW whose partition range is a subset with the
    same source is redundant (covers identical APs too).  Conservative: LdWs
    carrying sync waits/updates are kept; any instruction with unknown PE-array
    effect resets tracking.
    """
    removed = 0
    for blk in nc.m.functions[0].blocks:
        insts = list(blk.instructions)
        if not any(type(i).__name__ == "InstLdweights" for i in insts):
            continue
        prev = None
        to_remove = []
        for inst in insts:
            nm = type(inst).__name__
            if nm == "InstLdweights":
                d = _ldw_desc(inst)
                si = inst.sync_info
                clean = not si or (not si.on_wait and not si.on_update)
                if (d is not None and prev is not None and clean
                        and d[0] == prev[0] and d[1] == prev[1]
                        and d[2] == prev[2] and d[5] == prev[5]
                        and d[6] == prev[6]
                        and d[3] >= prev[3]
                        and d[3] + d[4] <= prev[3] + prev[4]):
                    to_remove.append(inst)
                else:
                    prev = d
            elif nm == "InstMatmult":
                pass
            elif nm in ("InstEventSemaphore", "InstNop", "InstTensorLoad",
                        "InstTensorSave"):
                pass
            else:
                prev = None
        for inst in to_remove:
            blk.instructions.remove(inst)
            removed += 1
    return removed


def prep_inputs(x, weight, bias):
    """Host-side shard + relayout + bf16 cast. Returns in_maps for 8 cores."""
    x = np.asarray(x, dtype=np.float32)
    weight = np.asarray(weight, dtype=np.float32)
    bias = np.asarray(bias, dtype=np.float32)

    # (w, t) -> j / kw maps for the diagonal layout
    jm = (np.arange(W)[:, None] - 2) + np.arange(K)[None, :]      # [W, K]
    val = (jm >= 0) & (jm < OH)
    jc = np.where(val, jm, 0)
    kwm = np.broadcast_to(2 - np.arange(K)[None, :], (W, K))      # [W, K]

    in_maps = []
    for r0 in ROW0:
        # ---- x tile: [c2=128, slab(5), w, b] bf16
        xr = np.zeros((C, 7, W, B), dtype=np.float32)
        n = min(7, H - r0)
        xr[:, :n] = x[:, :, r0:r0 + n, :].transpose(1, 2, 3, 0)
        top = xr[:, SLABS]                     # [C, 5, W, B]
        bot = xr[:, [s + 1 for s in SLABS]]
        xt = np.concatenate([top, bot], axis=0).astype(_BF16)

        # ---- weight chunks: weight[c, o, i, j, kh, kw], rows i = r0..r0+3
        Wc = weight[:, :, r0:r0 + R]           # [C, O, R, OW, K, K]
        # Bkh[kh][c, o, i, w, t] = W[c,o,i, j(w,t), kh, kw(t)] * valid
        Bkh = []
        for kh in range(K):
            S = Wc[:, :, :, :, kh, :]          # [C, O, R, OW, K(kw)]
            g = S[:, :, :, jc, kwm]            # [C, O, R, W, K(t)]
            g = g * val[None, None, None]
            Bkh.append(g)
        # pair chunks: [c2, i, w, t, o]: top=kh0, bottom=kh1
        pair = np.concatenate([Bkh[0], Bkh[1]], axis=0)   # [128, O, R, W, K]
        pair = pair.transpose(0, 2, 3, 4, 1)               # [128, R, W, K, O]
        pair = pair[:, :, val, :].astype(_BF16)            # packed: [128, R, 90, O]
        wpc = np.ascontiguousarray(pair.reshape(B, R * WCH))
        # singles: kh=2 for row i -> Bkh[2][:, :, i]
        # ws0: partitions 0-63 = single(i=1), 64-127 = single(i=0)
        # ws1: partitions 0-63 = single(i=3), 64-127 = single(i=2)
        s = Bkh[2].transpose(0, 2, 3, 4, 1)    # [C, R, W, K, O]
        ws0 = np.concatenate([s[:, 1], s[:, 0]], axis=0)   # [128, W, K, O]
        ws1 = np.concatenate([s[:, 3], s[:, 2]], axis=0)
        wsc = np.stack([ws0[:, val, :], ws1[:, val, :]],
                       axis=1).astype(_BF16)               # [128, 2, 90, O]

        # ---- bias broadcast over b partitions: [b, i, j, o] f32
        bb = bias[:, r0:r0 + R, :].transpose(1, 2, 0)      # [R, OW, O]
        bb = np.broadcast_to(bb.reshape(1, OUT_FREE), (B, OUT_FREE))

        in_maps.append({
            "xt": np.ascontiguousarray(xt.reshape(B, XT_FREE)),
            "wp": wpc,
            "ws": np.ascontiguousarray(wsc.reshape(B, 2 * WCH)),
            "biasb": np.ascontiguousarray(bb, dtype=np.float32),
        })
    return in_maps


def gather_output(results):
    out = np.empty((B, O, OH, OW), dtype=np.float32)
    for k, r0 in enumerate(ROW0):
        co = results[k]["outp"].reshape(B, R, OW, O).astype(np.float32)
        lo = 0 if k < 7 else 2
        out[:, :, r0 + lo:r0 + R, :] = co[:, lo:].transpose(0, 3, 1, 2)
    return out


_NC_CACHE = {}


def kernel(x: np.ndarray, weight: np.ndarray, bias: np.ndarray) -> np.ndarray:
    if "nc" not in _NC_CACHE:
        _NC_CACHE["nc"] = build_nc()
    nc = _NC_CACHE["nc"]
    in_maps = prep_inputs(np.asarray(x), np.asarray(weight), np.asarray(bias))
    res = run_bass_kernel_spmd(nc, in_maps, core_ids=list(range(N_CORES)))
    return gather_output(res.results)
